# revision 17
# baseline (speedup 1.0000x reference)
"""NonLocalBlock (spatial self-attention) Trainium2 Bass kernel.

Problem: x [4, 128, 64, 64]; 1x1 convs theta/phi/g -> softmax(theta^T phi) g
-> 1x1 conv out + residual.

Sharding (8 cores): core k -> (batch b = k//2, query-half h = k%2).
Each core holds the full keys/values for its batch (xkv [128, 4096]) and
computes attention output for its 2048 queries (xq [128, 2048]).
1x1 conv weights are replicated.

Per-core kernel structure (all layouts channel-on-partition):
  theta = wT_t.T @ xq   [128c, 2048n]   (+bias via ACT on PSUM->SBUF copy)
  phi   = wT_p.T @ xkv  [128c, 4096m]   (+bias)
  gT    = xkv_chunk.T @ wT_g  -> [m=128, c=128] chunks (g bias folded into
          the output bias: attn_norm(g0+b) = attn_norm(g0) + b, so
          b_out' = b_out + w_out @ b_g, applied in the residual term)
  per 512-query block:
    S^T chunks [m=128, n=512] = phi_chunk.T @ theta_blk  (PSUM)
    P^T = exp(S^T)  (ACT, PSUM->SBUF; doubles as the transpose-free copy;
                     max-subtraction skipped: scores are O(24), safe in fp32)
    attn_unnorm [c, n] += gT_chunk.T @ P^T_chunk   (PSUM accumulate over m)
    den [1, n]  += ones.T @ P^T_chunk              (PSUM accumulate over m)
    recip = 1/den; bc [128, n] = DMA-broadcast of recip over partitions
    attn = attn_unnorm * bc;  out = wT_o.T @ attn + (xq + b_out')  -> DRAM

QK runs with float32r operands and PV/den with bfloat16 (both ~4x the fp32
PE rate; fp32r keeps ~tf32 mantissa where score precision matters most).
The producing ACT/DVE ops write those dtypes so the BIR verifier's rounding
requirement is met. Projections and the epilogue are exact fp32. Block
epilogues are software-pipelined into the next block (PE never waits on the
DVE reciprocal chain), and QK/exp of pair p is interleaved with PV+den of
pair p-2 so PE and ACT stream without stalls.
"""

import numpy as np

B, C = 4, 128
HW = 4096  # 64*64 spatial positions
QH = HW // 2  # queries per core
NCORES = 8
NBLK = 512  # query block size
NMCH = HW // 128  # 32 key chunks of 128

# Attention matmul operand dtypes (QK scores; PV+den). float32r and
# bfloat16 run the PE at ~4x the fp32 rate; float32 is exact.
# Note: QK and PV both float32r faults the exec unit on this runtime
# (works individually — some fp32r scheduling erratum), so PV uses bf16.

_CACHE = {}


def _legalize_waits(bir, verbose=False):
    """Split instructions carrying more sync waits than the gen3 ISA allows.

    Walrus caps sync waits at 1 per instruction (2 for EventSemaphore); the
    Tile tail drain and first-consumer instructions can exceed that. Spill
    excess waits onto inserted wait-only EventSemaphore instructions placed
    immediately before the offender on the same engine (engines execute
    in order, so this is semantics-preserving).
    """
    n_split = 0
    where = []
    for f in bir["functions"]:
        for bb in f["blocks"]:
            out = []
            for inst in bb["instructions"]:
                si = inst.get("sync_info")
                waits = (si or {}).get("on_wait") or []
                cap = 2 if inst["opcode"] == "EventSemaphore" else 1
                if len(waits) > cap:
                    excess = waits[:-cap]
                    si["on_wait"] = waits[-cap:]
                    for i in range(0, len(excess), 2):
                        chunk = excess[i : i + 2]
                        out.append(
                            {
                                "debug": inst.get("debug", 0),
                                "engine": inst["engine"],
                                "ins": [],
                                "name": f'{inst["name"]}_w{i}',
                                "opcode": "EventSemaphore",
                                "outs": [],
                                "sync_info": {"on_update": [], "on_wait": chunk},
                            }
                        )
                        n_split += 1
                    where.append((inst["name"], inst["opcode"], len(excess)))
                out.append(inst)
            bb["instructions"] = out
    if verbose and where:
        print(f"[legalize_waits] {n_split} wait insts inserted for:")
        for nm, op, ne in where:
            print(f"  {nm} ({op}): {ne} excess waits")
    return bir


def _build(qk_dt="float32r", pv_dt="bfloat16"):
    from contextlib import ExitStack

    import concourse.bass as bass
    import concourse.tile as tile
    from concourse import mybir

    f32 = mybir.dt.float32
    dtmap = {
        "float32": f32,
        "float32r": mybir.dt.float32r,
        "bfloat16": mybir.dt.bfloat16,
    }
    fr_qk = dtmap[qk_dt]
    fr_pv = dtmap[pv_dt]

    Ident = mybir.ActivationFunctionType.Identity
    Exp = mybir.ActivationFunctionType.Exp

    nc = bass.Bass()
    x_kv = nc.dram_tensor("xkv", [C, HW], f32, kind="ExternalInput")
    x_q = nc.dram_tensor("xq", [C, QH], f32, kind="ExternalInput")
    w_d = {
        nm: nc.dram_tensor(nm, [C, C], f32, kind="ExternalInput")
        for nm in ("wt", "wp", "wg", "wo")
    }
    b_d = {
        nm: nc.dram_tensor(nm, [C, 1], f32, kind="ExternalInput")
        for nm in ("bt", "bp", "bg", "bo")
    }
    out_d = nc.dram_tensor("out", [C, QH], f32, kind="ExternalOutput")
    bc_d = nc.dram_tensor("bcrow", [QH // NBLK, NBLK], f32, kind="Internal")

    with ExitStack() as ctx:
        tc = ctx.enter_context(tile.TileContext(nc))
        const = ctx.enter_context(tc.tile_pool(name="const", bufs=1))
        persist = ctx.enter_context(tc.tile_pool(name="persist", bufs=1))
        small = ctx.enter_context(tc.tile_pool(name="small", bufs=2))
        pt_pool = ctx.enter_context(tc.tile_pool(name="pt", bufs=16))

        # ---- loads: small weights/biases first, then x in chunks so the
        # first projection matmuls start after ~one chunk of DMA ----
        w_s = {}
        for nm in ("wt", "wp", "wg", "wo"):
            t = const.tile([C, C], f32, tag=nm)
            nc.sync.dma_start(out=t, in_=w_d[nm][:, :])
            w_s[nm] = t
        b_s = {}
        for nm in ("bt", "bp", "bg", "bo"):
            t = const.tile([C, 1], f32, tag=nm)
            nc.sync.dma_start(out=t, in_=b_d[nm][:, :])
            b_s[nm] = t
        xkv_s = persist.tile([C, HW], f32, tag="xkv")
        for j in range(HW // 512):
            eng = nc.sync if j % 2 == 0 else nc.scalar
            eng.dma_start(
                out=xkv_s[:, j * 512 : (j + 1) * 512],
                in_=x_kv[:, j * 512 : (j + 1) * 512],
            )
        xq_s = persist.tile([C, QH], f32, tag="xq")
        for j in range(QH // 512):
            eng = nc.sync if j % 2 == 0 else nc.scalar
            eng.dma_start(
                out=xq_s[:, j * 512 : (j + 1) * 512],
                in_=x_q[:, j * 512 : (j + 1) * 512],
            )
        ones_f32 = const.tile([128, 1], f32, tag="ones_f32")
        nc.vector.memset(ones_f32, 1.0)
        ones_col = const.tile([128, 1], fr_pv, tag="ones_col")
        nc.vector.tensor_copy(out=ones_col, in_=ones_f32)

        theta_s = persist.tile([C, QH], fr_qk, tag="theta")
        phi_s = persist.tile([C, HW], fr_qk, tag="phi")
        gT_s = persist.tile([128, NMCH, 128], fr_pv, tag="gT")
        xqb_s = persist.tile([C, QH], f32, tag="xqb")

        # ---- projections (exact fp32 matmuls; outputs rounded to fr) ----
        with tc.tile_pool(name="proj_ps", bufs=4, space="PSUM") as proj_ps:
            for j in range(HW // 512):  # phi
                ps = proj_ps.tile([128, 512], f32, tag="p")
                nc.tensor.matmul(
                    ps,
                    w_s["wp"],
                    xkv_s[:, j * 512 : (j + 1) * 512],
                    start=True,
                    stop=True,
                )
                nc.scalar.activation(
                    out=phi_s[:, j * 512 : (j + 1) * 512],
                    in_=ps,
                    func=Ident,
                    bias=b_s["bp"],
                    scale=1.0,
                )
            for j in range(QH // 512):  # theta
                ps = proj_ps.tile([128, 512], f32, tag="p")
                nc.tensor.matmul(
                    ps,
                    w_s["wt"],
                    xq_s[:, j * 512 : (j + 1) * 512],
                    start=True,
                    stop=True,
                )
                nc.scalar.activation(
                    out=theta_s[:, j * 512 : (j + 1) * 512],
                    in_=ps,
                    func=Ident,
                    bias=b_s["bt"],
                    scale=1.0,
                )
            # gT chunks: gT[m, c] = xkv_chunk.T @ w_gT (bias via b_out')
            for mi in range(NMCH):
                ps = proj_ps.tile([128, 128], f32, tag="p")
                nc.tensor.matmul(
                    ps,
                    xkv_s[:, mi * 128 : (mi + 1) * 128],
                    w_s["wg"],
                    start=True,
                    stop=True,
                )
                if mi % 2 == 0:
                    nc.vector.tensor_copy(out=gT_s[:, mi, :], in_=ps)
                else:
                    nc.scalar.copy(out=gT_s[:, mi, :], in_=ps)
            # combined output bias: b_out' = b_out + w_out @ b_g
            psb = proj_ps.tile([128, 1], f32, tag="p")
            nc.tensor.matmul(psb, w_s["wo"], b_s["bg"], start=True, stop=True)
            bcomb_s = const.tile([C, 1], f32, tag="bcomb")
            nc.scalar.activation(
                out=bcomb_s, in_=psb, func=Ident, bias=b_s["bo"], scale=1.0
            )
            # xqb = xq + b_out'
            for j in range(QH // 512):
                nc.scalar.activation(
                    out=xqb_s[:, j * 512 : (j + 1) * 512],
                    in_=xq_s[:, j * 512 : (j + 1) * 512],
                    func=Ident,
                    bias=bcomb_s,
                    scale=1.0,
                )

        # ---- attention ----
        s_pool = ctx.enter_context(tc.tile_pool(name="s_ps", bufs=2, space="PSUM"))
        attn_pool = ctx.enter_context(tc.tile_pool(name="attn_ps", bufs=2, space="PSUM"))
        den_pool = ctx.enter_context(tc.tile_pool(name="den_ps", bufs=1, space="PSUM"))
        conv_pool = ctx.enter_context(tc.tile_pool(name="conv_ps", bufs=1, space="PSUM"))

        # Software-pipelined blocks: block b's normalization/conv epilogue is
        # emitted during block b+1 so the PE never stalls on the DVE chain
        # (reciprocal of [1,512] alone is ~3.3us) and HAM stays warm.
        pending = None  # (attn_ps, bc_s, q0) of the previous block

        def finish_block(attn_ps, bc_s, q0):
            attn_s = small.tile([128, 512], f32, tag="attn_s")
            nc.vector.tensor_mul(attn_s, attn_ps, bc_s)
            conv_ps = conv_pool.tile([128, 512], f32, tag="conv")
            nc.tensor.matmul(conv_ps, w_s["wo"], attn_s, start=True, stop=True)
            out_s = small.tile([128, 512], f32, tag="out_s")
            nc.vector.tensor_add(out_s, conv_ps, xqb_s[:, q0 : q0 + NBLK])
            nc.sync.dma_start(out=out_d[:, q0 : q0 + NBLK], in_=out_s)

        NPAIR = NMCH // 2
        for blk in range(QH // NBLK):
            q0 = blk * NBLK
            thq = theta_s[:, q0 : q0 + NBLK]
            pt_tiles = []
            ptsum_tiles = []
            attn_ps = attn_pool.tile([128, 512], f32, tag="attn")
            den_ps = den_pool.tile([1, 512], f32, tag="den")
            # Interleave score/exp pair pj with PV+den of pair pj-2: the PE
            # stream stays dense (QK pair + PV pair + den pair per step
            # exceeds the ACT exp-pair latency, so neither engine stalls).
            for pj in range(NPAIR + 2):
                if pj < NPAIR:
                    sp = s_pool.tile([128, 2, 512], f32, tag="s")
                    for k2 in range(2):
                        mi = pj * 2 + k2
                        nc.tensor.matmul(
                            sp[:, k2, :],
                            phi_s[:, mi * 128 : (mi + 1) * 128],
                            thq,
                            start=True,
                            stop=True,
                        )
                    pt = pt_pool.tile([128, 2, 512], fr_pv, tag="pt")
                    nc.scalar.activation(
                        out=pt, in_=sp, func=Exp, bias=0.0, scale=1.0
                    )
                    pt_tiles.append(pt)
                    # pair-sum on DVE so the den matmul count halves (the
                    # bf16 pair-sum roundings average out across 2048 pairs)
                    pts = pt_pool.tile([128, 512], fr_pv, tag="ptsum")
                    nc.vector.tensor_add(pts, pt[:, 0, :], pt[:, 1, :])
                    ptsum_tiles.append(pts)
                if pj == 7 and pending is not None:
                    finish_block(*pending)
                if pj >= 2:
                    p = pj - 2
                    for k2 in range(2):
                        mi = p * 2 + k2
                        nc.tensor.matmul(
                            attn_ps,
                            gT_s[:, mi, :],
                            pt_tiles[p][:, k2, :],
                            start=(mi == 0),
                            stop=(mi == NMCH - 1),
                        )
                    nc.tensor.matmul(
                        den_ps,
                        ones_col,
                        ptsum_tiles[p],
                        start=(p == 0),
                        stop=(p == NPAIR - 1),
                    )
            recip_row = small.tile([1, 512], f32, tag="recip")
            nc.vector.reciprocal(out=recip_row, in_=den_ps)
            nc.sync.dma_start(out=bc_d[blk : blk + 1, :], in_=recip_row)
            bc_s = small.tile([128, 512], f32, tag="bc_s")
            nc.sync.dma_start(out=bc_s, in_=bc_d[blk].partition_broadcast(128))
            pending = (attn_ps, bc_s, q0)
        finish_block(*pending)

    import json as _json
    import os as _os

    blob = _json.dumps(
        _legalize_waits(
            _json.loads(nc.to_json_bytes()),
            verbose=bool(_os.environ.get("KERNEL_DEBUG")),
        )
    ).encode()
    nc.to_json_bytes = lambda: blob
    return nc


QK_DT = "float32r"
PV_DT = "bfloat16"


def _get_nc():
    key = (QK_DT, PV_DT)
    if key not in _CACHE:
        _CACHE[key] = _build(*key)
    return _CACHE[key]


def _run(inputs, trace=False, **spmd_kwargs):
    from concourse.bass_utils import run_bass_kernel_spmd

    x = np.asarray(inputs["x"], np.float32)
    xf = np.ascontiguousarray(x.reshape(B, C, HW))
    wT = {
        "wt": np.ascontiguousarray(np.asarray(inputs["w_theta"], np.float32).T),
        "wp": np.ascontiguousarray(np.asarray(inputs["w_phi"], np.float32).T),
        "wg": np.ascontiguousarray(np.asarray(inputs["w_g"], np.float32).T),
        "wo": np.ascontiguousarray(np.asarray(inputs["w_out"], np.float32).T),
    }
    bcol = {
        "bt": np.ascontiguousarray(np.asarray(inputs["b_theta"], np.float32).reshape(C, 1)),
        "bp": np.ascontiguousarray(np.asarray(inputs["b_phi"], np.float32).reshape(C, 1)),
        "bg": np.ascontiguousarray(np.asarray(inputs["b_g"], np.float32).reshape(C, 1)),
        "bo": np.ascontiguousarray(np.asarray(inputs["b_out"], np.float32).reshape(C, 1)),
    }
    in_maps = []
    for k in range(NCORES):
        b, h = k // 2, k % 2
        in_maps.append(
            {
                "xkv": xf[b],
                "xq": np.ascontiguousarray(xf[b][:, h * QH : (h + 1) * QH]),
                **wT,
                **bcol,
            }
        )
    nc = _get_nc()
    res = run_bass_kernel_spmd(
        nc, in_maps, core_ids=list(range(NCORES)), trace=trace, **spmd_kwargs
    )
    out = np.empty((B, C, HW), np.float32)
    for k in range(NCORES):
        b, h = k // 2, k % 2
        out[b][:, h * QH : (h + 1) * QH] = res.results[k]["out"]
    return out.reshape(B, C, 64, 64), res


def kernel(**inputs):
    out, _ = _run(inputs, trace=False)
    return out


# revision 19
# speedup vs baseline: 1.0248x; 1.0248x over previous
"""NonLocalBlock (spatial self-attention) Trainium2 Bass kernel.

Problem: x [4, 128, 64, 64]; 1x1 convs theta/phi/g -> softmax(theta^T phi) g
-> 1x1 conv out + residual.

Sharding (8 cores): core k -> (batch b = k//2, query-half h = k%2).
Each core holds the full keys/values for its batch (xkv [128, 4096]) and
computes attention output for its 2048 queries (xq [128, 2048]).
1x1 conv weights are replicated.

Per-core kernel structure (all layouts channel-on-partition):
  theta = wT_t.T @ xq   [128c, 2048n]   (+bias via ACT on PSUM->SBUF copy)
  phi   = wT_p.T @ xkv  [128c, 4096m]   (+bias)
  gT    = xkv_chunk.T @ wT_g  -> [m=128, c=128] chunks (g bias folded into
          the output bias: attn_norm(g0+b) = attn_norm(g0) + b, so
          b_out' = b_out + w_out @ b_g, applied in the residual term)
  per 512-query block:
    S^T chunks [m=128, n=512] = phi_chunk.T @ theta_blk  (PSUM)
    P^T = exp(S^T)  (ACT, PSUM->SBUF; doubles as the transpose-free copy;
                     max-subtraction skipped: scores are O(24), safe in fp32)
    attn_unnorm [c, n] += gT_chunk.T @ P^T_chunk   (PSUM accumulate over m)
    den [1, n]  += ones.T @ P^T_chunk              (PSUM accumulate over m)
    recip = 1/den; bc [128, n] = DMA-broadcast of recip over partitions
    attn = attn_unnorm * bc;  out = wT_o.T @ attn + (xq + b_out')  -> DRAM

QK runs with float32r operands and PV/den with bfloat16 (both ~4x the fp32
PE rate; fp32r keeps ~tf32 mantissa where score precision matters most).
The producing ACT/DVE ops write those dtypes so the BIR verifier's rounding
requirement is met. Projections and the epilogue are exact fp32. Block
epilogues are software-pipelined into the next block (PE never waits on the
DVE reciprocal chain), and QK/exp of pair p is interleaved with PV+den of
pair p-2 so PE and ACT stream without stalls.
"""

import numpy as np

B, C = 4, 128
HW = 4096  # 64*64 spatial positions
QH = HW // 2  # queries per core
NCORES = 8
NBLK = 512  # query block size
NMCH = HW // 128  # 32 key chunks of 128

# Attention matmul operand dtypes (QK scores; PV+den). float32r and
# bfloat16 run the PE at ~4x the fp32 rate; float32 is exact.
# Note: QK and PV both float32r faults the exec unit on this runtime
# (works individually — some fp32r scheduling erratum), so PV uses bf16.

_CACHE = {}


def _legalize_waits(bir, verbose=False):
    """Split instructions carrying more sync waits than the gen3 ISA allows.

    Walrus caps sync waits at 1 per instruction (2 for EventSemaphore); the
    Tile tail drain and first-consumer instructions can exceed that. Spill
    excess waits onto inserted wait-only EventSemaphore instructions placed
    immediately before the offender on the same engine (engines execute
    in order, so this is semantics-preserving).
    """
    n_split = 0
    where = []
    for f in bir["functions"]:
        for bb in f["blocks"]:
            out = []
            for inst in bb["instructions"]:
                si = inst.get("sync_info")
                waits = (si or {}).get("on_wait") or []
                cap = 2 if inst["opcode"] == "EventSemaphore" else 1
                if len(waits) > cap:
                    excess = waits[:-cap]
                    si["on_wait"] = waits[-cap:]
                    for i in range(0, len(excess), 2):
                        chunk = excess[i : i + 2]
                        out.append(
                            {
                                "debug": inst.get("debug", 0),
                                "engine": inst["engine"],
                                "ins": [],
                                "name": f'{inst["name"]}_w{i}',
                                "opcode": "EventSemaphore",
                                "outs": [],
                                "sync_info": {"on_update": [], "on_wait": chunk},
                            }
                        )
                        n_split += 1
                    where.append((inst["name"], inst["opcode"], len(excess)))
                out.append(inst)
            bb["instructions"] = out
    if verbose and where:
        print(f"[legalize_waits] {n_split} wait insts inserted for:")
        for nm, op, ne in where:
            print(f"  {nm} ({op}): {ne} excess waits")
    return bir


def _build(qk_dt="float32r", pv_dt="bfloat16"):
    from contextlib import ExitStack

    import concourse.bass as bass
    import concourse.tile as tile
    from concourse import mybir

    f32 = mybir.dt.float32
    dtmap = {
        "float32": f32,
        "float32r": mybir.dt.float32r,
        "bfloat16": mybir.dt.bfloat16,
    }
    fr_qk = dtmap[qk_dt]
    fr_pv = dtmap[pv_dt]

    Ident = mybir.ActivationFunctionType.Identity
    Exp = mybir.ActivationFunctionType.Exp

    nc = bass.Bass()
    x_kv = nc.dram_tensor("xkv", [C, HW], f32, kind="ExternalInput")
    x_q = nc.dram_tensor("xq", [C, QH], f32, kind="ExternalInput")
    w_d = {
        nm: nc.dram_tensor(nm, [C, C], f32, kind="ExternalInput")
        for nm in ("wt", "wp", "wg", "wo")
    }
    b_d = {
        nm: nc.dram_tensor(nm, [C, 1], f32, kind="ExternalInput")
        for nm in ("bt", "bp", "bg", "bo")
    }
    out_d = nc.dram_tensor("out", [C, QH], f32, kind="ExternalOutput")
    bc_d = nc.dram_tensor("bcrow", [QH // NBLK, NBLK], f32, kind="Internal")

    with ExitStack() as ctx:
        tc = ctx.enter_context(tile.TileContext(nc))
        const = ctx.enter_context(tc.tile_pool(name="const", bufs=1))
        persist = ctx.enter_context(tc.tile_pool(name="persist", bufs=1))
        small = ctx.enter_context(tc.tile_pool(name="small", bufs=2))
        pt_pool = ctx.enter_context(tc.tile_pool(name="pt", bufs=16))

        # ---- loads: small weights/biases first, then x in chunks so the
        # first projection matmuls start after ~one chunk of DMA ----
        w_s = {}
        for nm in ("wt", "wp", "wg", "wo"):
            t = const.tile([C, C], f32, tag=nm)
            nc.sync.dma_start(out=t, in_=w_d[nm][:, :])
            w_s[nm] = t
        b_s = {}
        for nm in ("bt", "bp", "bg", "bo"):
            t = const.tile([C, 1], f32, tag=nm)
            nc.sync.dma_start(out=t, in_=b_d[nm][:, :])
            b_s[nm] = t
        xq_s = persist.tile([C, QH], f32, tag="xq")
        for j in range(QH // 512):
            nc.sync.dma_start(
                out=xq_s[:, j * 512 : (j + 1) * 512],
                in_=x_q[:, j * 512 : (j + 1) * 512],
            )
        xkv_s = persist.tile([C, HW], f32, tag="xkv")
        for j in range(HW // 512):
            nc.sync.dma_start(
                out=xkv_s[:, j * 512 : (j + 1) * 512],
                in_=x_kv[:, j * 512 : (j + 1) * 512],
            )
        ones_f32 = const.tile([128, 1], f32, tag="ones_f32")
        nc.vector.memset(ones_f32, 1.0)
        ones_col = const.tile([128, 1], fr_pv, tag="ones_col")
        nc.vector.tensor_copy(out=ones_col, in_=ones_f32)

        theta_s = persist.tile([C, QH], fr_qk, tag="theta")
        phi_s = persist.tile([C, HW], fr_qk, tag="phi")
        gT_s = persist.tile([128, NMCH, 128], fr_pv, tag="gT")
        xqb_s = persist.tile([C, QH], f32, tag="xqb")

        # ---- projections (exact fp32 matmuls; outputs rounded to fr) ----
        with tc.tile_pool(name="proj_ps", bufs=4, space="PSUM") as proj_ps:
            for j in range(QH // 512):  # theta
                ps = proj_ps.tile([128, 512], f32, tag="p")
                nc.tensor.matmul(
                    ps,
                    w_s["wt"],
                    xq_s[:, j * 512 : (j + 1) * 512],
                    start=True,
                    stop=True,
                )
                nc.scalar.activation(
                    out=theta_s[:, j * 512 : (j + 1) * 512],
                    in_=ps,
                    func=Ident,
                    bias=b_s["bt"],
                    scale=1.0,
                )
            for j in range(HW // 512):  # phi
                ps = proj_ps.tile([128, 512], f32, tag="p")
                nc.tensor.matmul(
                    ps,
                    w_s["wp"],
                    xkv_s[:, j * 512 : (j + 1) * 512],
                    start=True,
                    stop=True,
                )
                nc.scalar.activation(
                    out=phi_s[:, j * 512 : (j + 1) * 512],
                    in_=ps,
                    func=Ident,
                    bias=b_s["bp"],
                    scale=1.0,
                )
            # gT chunks: gT[m, c] = xkv_chunk.T @ w_gT (bias via b_out')
            for mi in range(NMCH):
                ps = proj_ps.tile([128, 128], f32, tag="p")
                nc.tensor.matmul(
                    ps,
                    xkv_s[:, mi * 128 : (mi + 1) * 128],
                    w_s["wg"],
                    start=True,
                    stop=True,
                )
                if mi % 2 == 0:
                    nc.vector.tensor_copy(out=gT_s[:, mi, :], in_=ps)
                else:
                    nc.scalar.copy(out=gT_s[:, mi, :], in_=ps)
            # combined output bias: b_out' = b_out + w_out @ b_g
            psb = proj_ps.tile([128, 1], f32, tag="p")
            nc.tensor.matmul(psb, w_s["wo"], b_s["bg"], start=True, stop=True)
            bcomb_s = const.tile([C, 1], f32, tag="bcomb")
            nc.scalar.activation(
                out=bcomb_s, in_=psb, func=Ident, bias=b_s["bo"], scale=1.0
            )
            # xqb = xq + b_out'
            for j in range(QH // 512):
                nc.scalar.activation(
                    out=xqb_s[:, j * 512 : (j + 1) * 512],
                    in_=xq_s[:, j * 512 : (j + 1) * 512],
                    func=Ident,
                    bias=bcomb_s,
                    scale=1.0,
                )

        # ---- attention ----
        s_pool = ctx.enter_context(tc.tile_pool(name="s_ps", bufs=2, space="PSUM"))
        attn_pool = ctx.enter_context(tc.tile_pool(name="attn_ps", bufs=2, space="PSUM"))
        den_pool = ctx.enter_context(tc.tile_pool(name="den_ps", bufs=1, space="PSUM"))
        conv_pool = ctx.enter_context(tc.tile_pool(name="conv_ps", bufs=1, space="PSUM"))

        # Software-pipelined blocks: block b's normalization/conv epilogue is
        # emitted during block b+1 so the PE never stalls on the DVE chain
        # (reciprocal of [1,512] alone is ~3.3us) and HAM stays warm.
        pending = None  # (attn_ps, bc_s, q0) of the previous block

        def finish_block(attn_ps, bc_s, q0):
            attn_s = small.tile([128, 512], f32, tag="attn_s")
            nc.vector.tensor_mul(attn_s, attn_ps, bc_s)
            conv_ps = conv_pool.tile([128, 512], f32, tag="conv")
            nc.tensor.matmul(conv_ps, w_s["wo"], attn_s, start=True, stop=True)
            out_s = small.tile([128, 512], f32, tag="out_s")
            nc.vector.tensor_add(out_s, conv_ps, xqb_s[:, q0 : q0 + NBLK])
            nc.sync.dma_start(out=out_d[:, q0 : q0 + NBLK], in_=out_s)

        NPAIR = NMCH // 2
        for blk in range(QH // NBLK):
            q0 = blk * NBLK
            thq = theta_s[:, q0 : q0 + NBLK]
            pt_tiles = []
            ptsum_tiles = []
            attn_ps = attn_pool.tile([128, 512], f32, tag="attn")
            den_ps = den_pool.tile([1, 512], f32, tag="den")
            # Interleave score/exp pair pj with PV+den of pair pj-2: the PE
            # stream stays dense (QK pair + PV pair + den pair per step
            # exceeds the ACT exp-pair latency, so neither engine stalls).
            for pj in range(NPAIR + 2):
                if pj < NPAIR:
                    sp = s_pool.tile([128, 2, 512], f32, tag="s")
                    for k2 in range(2):
                        mi = pj * 2 + k2
                        nc.tensor.matmul(
                            sp[:, k2, :],
                            phi_s[:, mi * 128 : (mi + 1) * 128],
                            thq,
                            start=True,
                            stop=True,
                        )
                    pt = pt_pool.tile([128, 2, 512], fr_pv, tag="pt")
                    nc.scalar.activation(
                        out=pt, in_=sp, func=Exp, bias=0.0, scale=1.0
                    )
                    pt_tiles.append(pt)
                    # pair-sum on DVE so the den matmul count halves (the
                    # bf16 pair-sum roundings average out across 2048 pairs)
                    pts = pt_pool.tile([128, 512], fr_pv, tag="ptsum")
                    nc.vector.tensor_add(pts, pt[:, 0, :], pt[:, 1, :])
                    ptsum_tiles.append(pts)
                if pj == 7 and pending is not None:
                    finish_block(*pending)
                if 1 <= pj <= NPAIR:
                    d = pj - 1
                    nc.tensor.matmul(
                        den_ps,
                        ones_col,
                        ptsum_tiles[d],
                        start=(d == 0),
                        stop=(d == NPAIR - 1),
                    )
                if pj >= 2:
                    p = pj - 2
                    for k2 in range(2):
                        mi = p * 2 + k2
                        nc.tensor.matmul(
                            attn_ps,
                            gT_s[:, mi, :],
                            pt_tiles[p][:, k2, :],
                            start=(mi == 0),
                            stop=(mi == NMCH - 1),
                        )
            recip_row = small.tile([1, 512], f32, tag="recip")
            nc.vector.reciprocal(out=recip_row, in_=den_ps)
            nc.sync.dma_start(out=bc_d[blk : blk + 1, :], in_=recip_row)
            bc_s = small.tile([128, 512], f32, tag="bc_s")
            nc.sync.dma_start(out=bc_s, in_=bc_d[blk].partition_broadcast(128))
            pending = (attn_ps, bc_s, q0)
        finish_block(*pending)

    import json as _json
    import os as _os

    blob = _json.dumps(
        _legalize_waits(
            _json.loads(nc.to_json_bytes()),
            verbose=bool(_os.environ.get("KERNEL_DEBUG")),
        )
    ).encode()
    nc.to_json_bytes = lambda: blob
    return nc


QK_DT = "float32r"
PV_DT = "bfloat16"


def _get_nc():
    key = (QK_DT, PV_DT)
    if key not in _CACHE:
        _CACHE[key] = _build(*key)
    return _CACHE[key]


def _run(inputs, trace=False, **spmd_kwargs):
    from concourse.bass_utils import run_bass_kernel_spmd

    x = np.asarray(inputs["x"], np.float32)
    xf = np.ascontiguousarray(x.reshape(B, C, HW))
    wT = {
        "wt": np.ascontiguousarray(np.asarray(inputs["w_theta"], np.float32).T),
        "wp": np.ascontiguousarray(np.asarray(inputs["w_phi"], np.float32).T),
        "wg": np.ascontiguousarray(np.asarray(inputs["w_g"], np.float32).T),
        "wo": np.ascontiguousarray(np.asarray(inputs["w_out"], np.float32).T),
    }
    bcol = {
        "bt": np.ascontiguousarray(np.asarray(inputs["b_theta"], np.float32).reshape(C, 1)),
        "bp": np.ascontiguousarray(np.asarray(inputs["b_phi"], np.float32).reshape(C, 1)),
        "bg": np.ascontiguousarray(np.asarray(inputs["b_g"], np.float32).reshape(C, 1)),
        "bo": np.ascontiguousarray(np.asarray(inputs["b_out"], np.float32).reshape(C, 1)),
    }
    in_maps = []
    for k in range(NCORES):
        b, h = k // 2, k % 2
        in_maps.append(
            {
                "xkv": xf[b],
                "xq": np.ascontiguousarray(xf[b][:, h * QH : (h + 1) * QH]),
                **wT,
                **bcol,
            }
        )
    nc = _get_nc()
    res = run_bass_kernel_spmd(
        nc, in_maps, core_ids=list(range(NCORES)), trace=trace, **spmd_kwargs
    )
    out = np.empty((B, C, HW), np.float32)
    for k in range(NCORES):
        b, h = k // 2, k % 2
        out[b][:, h * QH : (h + 1) * QH] = res.results[k]["out"]
    return out.reshape(B, C, 64, 64), res


def kernel(**inputs):
    out, _ = _run(inputs, trace=False)
    return out
